# revision 46
# baseline (speedup 1.0000x reference)
"""Trainium2 Bass kernel for nn_CudaRendererGpu (differentiable-renderer forward).

Strategy (8 NeuronCores, SPMD), v16:
  Faces and vertices are sharded 8 ways (core c owns faces [25000c, 25000(c+1))
  and verts [12500c, 12500(c+1))). All per-vertex INPUT data (positions of the
  corner vertices of each vertex's 8 adjacent faces) is expanded on the HOST
  into contiguous per-core streams, so vertex normals are computed with ZERO
  gather descriptors (gather descriptor generation on the GPSIMD Q7 cores,
  ~8ns/idx on 2-of-8 cores per SWDGE queue, is the machine bottleneck).
  Only two gather phases remain, both split into 2048-idx calls round-robined
  over all 4 SWDGE queues with deep buffering so four descriptor generators
  run concurrently:
    P4: vn at face corners from a 4-packed all-gathered vn table; the 4-way
        sub-slot select is one mul over a [6f,4s] strided view + one reduce.
    P5: pixel phase over the packed face table fpk (pos streamed from host,
        vn from P4), pixels sorted by (b, face id), KPX=4 pixels per
        256B descriptor; b0/b1 calls interleaved so each call's fpk row
        range rises monotonically (pipelines against P4 chunk completion).
  Pixel math fuses the 3x3 projection into one 9-wide mul against a
  host-tiled M-row + an innermost-3 reduce through an aliased [3*KPX,3]
  view. Input loads ride the sync queue; result writes ride the scalar
  queue so they never head-of-line block input prefetch.
  Host does index composition/permutation only; all float math and all
  device-computed-table data movement happens on device.
"""

import numpy as np

import concourse.bass as bass
import concourse.mybir as mybir
import concourse.tile as tile
from concourse import bacc
from concourse.bass_utils import run_bass_kernel_spmd

F32 = mybir.dt.float32
I16 = mybir.dt.int16
P = 128
NI = 2048                # idxs per dma_gather call (P4)
NI5 = 4096               # idxs per pixel-phase gather call
KPX = 4                  # pixels packed per pixel-phase descriptor
ACH = 20                 # phase-A chunk columns (20 cols = 2560 verts)


def _ceil128(x):
    return (x + 127) // 128 * 128


class Cfg:
    def __init__(self, B=2, C=4, H=512, W=512, V=100000, F=200000, A=8):
        self.B, self.C, self.H, self.W, self.V, self.F, self.A = B, C, H, W, V, F, A
        self.NBC = B * C
        self.FSHV = F // 8            # valid faces per shard
        self.VSHV = V // 8
        self.FSH = 25600
        self.VSH = 12800
        self.FCOLS = self.FSH // P    # 200
        self.VCOLS = self.VSH // P    # 100
        self.VN_ROWS_L = P * (self.VCOLS // 4)   # 3200 vn rows per core
        self.VN_ROWS = 8 * self.VN_ROWS_L        # 25600 global (int16-safe)


def _call_sizes(total, ni=NI):
    out = []
    left = total
    while left > 0:
        c = min(ni, left)
        out.append(c)
        left -= c
    return out


def _call_sizes5(total):
    return _call_sizes(total, NI5)


def _wrap16(idx):
    """[N] (N%16==0) int array -> dma_gather idx layout [128, N//16]."""
    w = idx.reshape(-1, 16).T.astype(np.int16)
    return np.tile(w, (8, 1))


def _pack_calls(idx16, sizes, ni=NI):
    wi = np.zeros((len(sizes), P, ni // 16), np.int16)
    base = 0
    for i, n in enumerate(sizes):
        wi[i, :, : n // 16] = _wrap16(idx16[base:base + n])
        base += n
    return wi


def _grid_masks(sub, vals, cols, nsub):
    """sub [N], vals [N] -> m [P, cols, nsub]; grid slot i=(p=i%128, c=i//128)."""
    m = np.zeros((P, cols, nsub), np.float32)
    sg = sub.reshape(cols, P).T          # [P, cols]
    vg = vals.reshape(cols, P).T
    for s in range(nsub):
        m[:, :, s] = np.where(sg == s, vg, 0.0)
    return m


def _vn_row_sub(cfg, v):
    """global vertex id -> (vn_tab row, sub) for the 4-packed vn table."""
    c = cfg
    sh, j = np.divmod(v, c.VSHV)
    p, col = j % P, j // P
    return sh * c.VN_ROWS_L + p * (c.VCOLS // 4) + (col >> 2), col & 3


def host_prep(cfg, inputs):
    c = cfg
    faces = np.asarray(inputs["faces"]).astype(np.int64)
    vaf = np.asarray(inputs["vert_adj_faces"]).astype(np.int64)
    vaw = np.asarray(inputs["vert_adj_weights"]).astype(np.float32)
    vpos = np.asarray(inputs["vertex_pos"]).astype(np.float32)
    intr = np.asarray(inputs["intrinsics"]).astype(np.float32)
    extr = np.asarray(inputs["extrinsics"]).astype(np.float32)
    iext = np.asarray(inputs["inverse_extrinsics"]).astype(np.float32)
    fid = np.asarray(inputs["face_id"]).astype(np.int64)
    bary = np.asarray(inputs["barycentrics"]).astype(np.float32)

    # ---- per-bc projection constants, streamed per pixel slot ----
    # [M rows, each tiled x3 over verts (27) | t (3) | origin (3)]
    cst33 = np.zeros((c.NBC, 33), np.float32)
    for b in range(c.B):
        for cam in range(c.C):
            M = intr[b, cam] @ extr[b, cam][:, :3]
            t = intr[b, cam] @ extr[b, cam][:, 3]
            col = iext[b, cam][:, 3]
            orig = (col / col[3])[:3]
            for i in range(3):
                cst33[b * c.C + cam, 9 * i:9 * i + 9] = np.tile(M[i], 3)
            cst33[b * c.C + cam, 27:30] = t
            cst33[b * c.C + cam, 30:33] = orig

    p4sz = _call_sizes(c.FSH)

    # ---- expanded corner-position streams (pure permutations of inputs) ----
    # phase A layout: [v0b0 v0b1 v1b0 v1b1 v2b0 v2b1] (vert-major, b inner)
    fc18 = np.empty((c.F, 18), np.float32)
    for k in range(3):
        fc18[:, 6 * k:6 * k + 3] = vpos[0, faces[:, k]]
        fc18[:, 6 * k + 3:6 * k + 6] = vpos[1, faces[:, k]]
    # fpk pos layout: [pos_b0 (v0xyz v1xyz v2xyz), pos_b1 (9)]
    fp18 = np.concatenate([vpos[0, faces].reshape(c.F, 9),
                           vpos[1, faces].reshape(c.F, 9)], 1)

    # ---- pixel phase: per-core routing, one face-sorted stream per core ----
    fimg = fid.reshape(c.NBC, -1)                      # [8, H*W]
    grp = []                                           # per (core, b)
    ndesc = np.zeros((8, c.B), np.int64)
    for core in range(8):
        ent = []
        for b in range(c.B):
            sels, js, bcs = [], [], []
            for cam in range(c.C):
                bc = b * c.C + cam
                f = fimg[bc]
                sel = np.where((f >= core * c.FSHV)
                               & (f < (core + 1) * c.FSHV))[0]
                sels.append(sel)
                js.append(f[sel] - core * c.FSHV)
                bcs.append(np.full(len(sel), bc, np.int64))
            sel = np.concatenate(sels)
            j = np.concatenate(js)
            bcv = np.concatenate(bcs)
            order = np.argsort(j, kind="stable")
            sel, j, bcv = sel[order], j[order], bcv[order]
            brk = np.nonzero(np.diff(j))[0] + 1
            starts = np.concatenate([[0], brk]).astype(np.int64)
            lens = np.diff(np.concatenate([starts, [len(j)]]))
            dper = -(-lens // KPX)
            ndesc[core, b] = dper.sum()
            ent.append((sel, j, bcv, starts, lens, dper))
        grp.append(ent)
    caps = [max(NI5, int(_ceil128(ndesc[:, b].max()))) for b in range(c.B)]
    seg_base = [0, caps[0]]
    ndesc_tot = caps[0] + caps[1]
    nd_cols = ndesc_tot // P
    p5szs = []                                         # (b, ni, d0)
    per_b = []
    for b in range(c.B):
        off = seg_base[b]
        ent = []
        for ni in _call_sizes5(caps[b]):
            ent.append((b, ni, off))
            off += ni
        per_b.append(ent)
    # interleave b0/b1 calls so each call's fpk row range (and thus its
    # dependency on P4 chunks) increases monotonically with emission order
    for i in range(max(len(e) for e in per_b)):
        for ent in per_b:
            if i < len(ent):
                p5szs.append(ent[i])

    percore = []
    meta_px = []
    for core in range(8):
        im = {}
        f0 = core * c.FSHV
        v0 = core * c.VSHV
        # ---- phase A: expanded adjacency corner positions + weights ----
        av = np.zeros((c.VSH, c.A), np.int64)
        wv = np.zeros((c.VSH, c.A), np.float32)
        av[:c.VSHV] = vaf[v0:v0 + c.VSHV]
        wv[:c.VSHV] = vaw[v0:v0 + c.VSHV]
        ex = np.zeros((c.VSH, 18, c.A), np.float32)
        ex[:c.VSHV] = fc18[av[:c.VSHV]].transpose(0, 2, 1)
        # grid slot i=(p=i%128, c=i//128): [VSH,18,A] -> [P, VCOLS, 18, A]
        im["exp"] = (ex.reshape(c.VCOLS, P, 18, c.A)
                     .transpose(1, 0, 2, 3).copy())
        im["wts"] = (wv.reshape(c.VCOLS, P, c.A)
                     .transpose(1, 0, 2).copy())
        # ---- fpos: per-face corner positions in fpk layout (cols 0:18) ----
        fp = np.zeros((c.FSH, 18), np.float32)
        fp[:c.FSHV] = fp18[f0:f0 + c.FSHV]
        im["fpos"] = fp.reshape(c.FCOLS, P, 18).transpose(1, 0, 2).copy()
        # ---- P4: vn at corners (4-packed table, select4) ----
        fk = np.zeros((c.FSH, 3), np.int64)
        fk[:c.FSHV] = faces[f0:f0 + c.FSHV]
        p4i, p4m = [], []
        for k in range(3):
            row, sub = _vn_row_sub(c, fk[:, k])
            p4i.append(_pack_calls(row.astype(np.int16), p4sz))
            p4m.append(_grid_masks(sub, np.ones(c.FSH, np.float32),
                                   c.FCOLS, 4))
        im["p4i"] = np.stack(p4i)
        im["p4m"] = np.stack(p4m)
        # ---- P5: pixel descriptors + per-pixel payload ----
        drows = np.zeros(ndesc_tot, np.int64)
        bw = np.zeros((ndesc_tot, KPX, 36), np.float32)
        bw[:, :, 0] = 1.0                  # pad slots: bary=[1,0,0] ...
        bw[:, :, 32] = 1.0                 # ... and t2=1 keep all math finite
        px_meta = []
        for b in range(c.B):
            sel, j, bcv, starts, lens, dper = grp[core][b]
            base = seg_base[b]
            nvalid = int(dper.sum())
            dr = np.repeat(j[starts], dper)
            drows[base:base + nvalid] = dr
            drows[base + nvalid:base + caps[b]] = dr[-1] if nvalid else 0
            pos_in_run = np.arange(len(j)) - np.repeat(starts, lens)
            dbase = np.concatenate([[0], np.cumsum(dper)])[:-1]
            didx = base + np.repeat(dbase, lens) + pos_in_run // KPX
            kk = pos_in_run % KPX
            bw[didx, kk, 0:3] = bary.reshape(c.NBC, -1, 3)[bcv, sel]
            bw[didx, kk, 3:36] = cst33[bcv]
            px_meta.append((sel, bcv, didx, kk))
        im["drows"] = drows                # int64, converted per-call below
        im["bw"] = bw.reshape(nd_cols, P, KPX, 36).transpose(1, 0, 2, 3).copy()
        percore.append(im)
        meta_px.append(px_meta)

    # per-call fpk row ranges unioned across cores, then relative idx streams
    p5_plan = []
    for b, ni, off in p5szs:
        r0 = min(int(im["drows"][off:off + ni].min()) for im in percore)
        r1 = max(int(im["drows"][off:off + ni].max()) for im in percore) + 1
        assert r1 - r0 < 32768
        p5_plan.append((b, ni, off, r0, r1))
    for im in percore:
        p5i = np.zeros((len(p5_plan), P, NI5 // 16), np.int16)
        for ci, (b, ni, d0, r0, r1) in enumerate(p5_plan):
            p5i[ci, :, : ni // 16] = _wrap16(im["drows"][d0:d0 + ni] - r0)
        im["p5i"] = p5i
        del im["drows"]

    meta = dict(p4sz=p4sz, p5_plan=p5_plan, nd_cols=nd_cols, px=meta_px)
    return percore, meta


def _mul(nc, out, a, b):
    nc.vector.tensor_tensor(out=out, in0=a, in1=b, op=mybir.AluOpType.mult)


def _add(nc, out, a, b):
    nc.vector.tensor_tensor(out=out, in0=a, in1=b, op=mybir.AluOpType.add)


def _sub(nc, out, a, b):
    nc.vector.tensor_tensor(out=out, in0=a, in1=b, op=mybir.AluOpType.subtract)


def build_program(cfg, meta, num_swdge_queues=4):
    c = cfg
    p4sz = meta["p4sz"]
    p5_plan = meta["p5_plan"]
    nd_cols = meta["nd_cols"]
    NQ = num_swdge_queues

    nc = bacc.Bacc("TRN2", target_bir_lowering=False, debug=False,
                   num_devices=8, num_swdge_queues=NQ,
                   dynamic_dma_scratch_size=32768)

    def din(name, shape, dt=F32):
        return nc.dram_tensor(name, list(shape), dt, kind="ExternalInput").ap()

    exp_in = din("exp", [P, c.VCOLS, 18, c.A])
    wts_in = din("wts", [P, c.VCOLS, c.A])
    fpos_in = din("fpos", [P, c.FCOLS, 18])
    p4i = din("p4i", [3, len(p4sz), P, NI // 16], I16)
    p4m = din("p4m", [3, P, c.FCOLS, 4])
    p5i = din("p5i", [len(p5_plan), P, NI5 // 16], I16)
    bw_in = din("bw", [P, nd_cols, KPX, 36])

    o_pos = nc.dram_tensor("o_pos", [P, nd_cols, KPX, 3], F32,
                           kind="ExternalOutput").ap()
    o_nrm = nc.dram_tensor("o_nrm", [P, nd_cols, KPX, 3], F32,
                           kind="ExternalOutput").ap()
    o_scr = nc.dram_tensor("o_scr", [P, nd_cols, KPX, 3], F32,
                           kind="ExternalOutput").ap()
    o_dep = nc.dram_tensor("o_dep", [P, nd_cols, KPX], F32,
                           kind="ExternalOutput").ap()

    groups = [list(range(8))]
    qctr = [0]

    def gather_prep(pool, tab_ap, idx_tile, ni, tag, cap=NI):
        g = pool.tile([P, cap // P, 64], F32, tag=tag, name=tag)
        q = qctr[0] % NQ
        nc.gpsimd.dma_gather(
            out_ap=g[:, : ni // P, :], in_ap=tab_ap, idxs_ap=idx_tile,
            num_idxs=ni, num_idxs_reg=ni, elem_size=64,
            single_packet=False, queue_num=q)
        qctr[0] += 1
        return g, q

    def fire(q):
        pass

    def select4(nc, tmp_pool, g, msk, out):
        """out[:, :, 0:6] = sum_s msk[..., s] * g[:, :, 16s:16s+6].

        One mul over a [6f, 4s] strided view (f stride 1, s stride 16) and
        one innermost-4 reduce."""
        nn = out.shape[1]
        gv = g[:, :nn, :].rearrange("p n (s f) -> p n f s", s=4)[:, :, 0:6, :]
        mb = msk[:, :nn, :].unsqueeze(2).to_broadcast([P, nn, 6, 4])
        t = tmp_pool.tile([P, NI // P, 6, 4], F32, tag="selt", bufs=2,
                          name="selt")[:, :nn]
        _mul(nc, t, gv, mb)
        nc.vector.tensor_reduce(out=out.unsqueeze(3), in_=t,
                                axis=mybir.AxisListType.X,
                                op=mybir.AluOpType.add)

    with tile.TileContext(nc) as tc:
        with tc.tile_pool(name="dram", bufs=1, space="DRAM") as dram, \
             tc.tile_pool(name="expp", bufs=2) as exp_pool, \
             tc.tile_pool(name="atmp", bufs=1) as atmp_pool, \
             tc.tile_pool(name="acc", bufs=1) as acc_pool, \
             tc.tile_pool(name="gath", bufs=6) as gath_pool, \
             tc.tile_pool(name="gath5", bufs=4) as gath5_pool, \
             tc.tile_pool(name="idx", bufs=16) as idx_pool, \
             tc.tile_pool(name="msk", bufs=10) as msk_pool, \
             tc.tile_pool(name="tmp", bufs=1) as tmp_pool, \
             tc.tile_pool(name="fpk", bufs=2) as fpk_pool, \
             tc.tile_pool(name="px", bufs=2) as px_pool:

            vn_tab = dram.tile([c.VN_ROWS, 64], F32, tag="vn_tab")
            vnb = dram.tile([P, c.VCOLS // 4, 64], F32, tag="vnb")
            fpk_d = dram.tile([c.FSH, 64], F32, tag="fpk")

            # ---------------- A: vertex normals, no gathers ----------------
            vna = acc_pool.tile([P, c.VCOLS, 6], F32, tag="vna", name="vna")
            for c0 in range(0, c.VCOLS, ACH):
                cs = slice(c0, c0 + ACH)
                ex = exp_pool.tile([P, ACH, 18, c.A], F32, tag="ex", name="ex")
                nc.sync.dma_start(ex[:], exp_in[:, cs, :, :])
                w = exp_pool.tile([P, ACH, c.A], F32, tag="w", name="w")
                nc.sync.dma_start(w[:], wts_in[:, cs, :])
                e1 = atmp_pool.tile([P, ACH, 6, c.A], F32, tag="e1", name="e1")
                e2 = atmp_pool.tile([P, ACH, 6, c.A], F32, tag="e2", name="e2")
                cr = atmp_pool.tile([P, ACH, 6, c.A], F32, tag="cr", name="cr")
                ct = atmp_pool.tile([P, ACH, 1, c.A], F32, tag="ct", name="ct")
                _sub(nc, e1[:], ex[:, :, 6:12, :], ex[:, :, 0:6, :])
                _sub(nc, e2[:], ex[:, :, 12:18, :], ex[:, :, 0:6, :])
                for b in range(2):
                    for i in range(3):
                        j, l = (i + 1) % 3, (i + 2) % 3
                        o = cr[:, :, 3 * b + i:3 * b + i + 1, :]
                        _mul(nc, o, e1[:, :, 3 * b + j:3 * b + j + 1, :],
                             e2[:, :, 3 * b + l:3 * b + l + 1, :])
                        _mul(nc, ct[:], e1[:, :, 3 * b + l:3 * b + l + 1, :],
                             e2[:, :, 3 * b + j:3 * b + j + 1, :])
                        _sub(nc, o, o, ct[:])
                wb = w[:, :, :].unsqueeze(2).to_broadcast([P, ACH, 6, c.A])
                _mul(nc, cr[:], cr[:], wb)
                nc.vector.tensor_reduce(out=vna[:, cs, :].unsqueeze(3),
                                        in_=cr[:], axis=mybir.AxisListType.X,
                                        op=mybir.AluOpType.add)

            # repack into 4-packed rows (AllGather emitted below, after the
            # first round of P4 preps so their desc-gen is not blocked
            # behind the collective in the gpsimd stream)
            vnb_s = acc_pool.tile([P, c.VCOLS // 4, 64], F32, tag="vnb_s",
                                  name="vnb_s")
            nc.vector.memset(vnb_s[:], 0.0)
            for s in range(4):
                nc.scalar.copy(out=vnb_s[:, :, 16 * s:16 * s + 6],
                               in_=vna[:, s::4, :])
            nc.scalar.dma_start(vnb[:], vnb_s[:])

            # ---------------- P4: fpk assembly ----------------
            # fpk row: [pos(18): v0b0 v0b1 v1b0 v1b1 v2b0 v2b1 | vn(18) | pad]
            # Emission in rounds of NQ: preps (desc-gen, no table dep), then
            # triggers (gated on vn_tab), then select4 consumers. Input loads
            # go on the sync queue; result writes go on the scalar queue so
            # they never head-of-line block input prefetch.
            fsb_tiles = {}


            def p4_consume(g, ci, k, ni):
                nn = ni // P
                base = sum(p4sz[:ci])
                cs = slice(base // P, (base + ni) // P)
                if ci not in fsb_tiles:
                    fsb_tiles[ci] = fpk_pool.tile([P, NI // P, 64], F32,
                                                  tag="fsb", name="fsb")
                    nc.sync.dma_start(fsb_tiles[ci][:, :nn, 0:18],
                                      fpos_in[:, cs, :])
                fsb = fsb_tiles[ci]
                mk = msk_pool.tile([P, NI // P, 4], F32)
                nc.sync.dma_start(mk[:, :nn, :], p4m[k, :, cs, :])
                # Tile's DMASW wait on a PREPARE_ONLY gather fires at
                # desc-gen, not DMA completion -- gate on the baked
                # per-queue sem (cumulative, in ring order).
                nc.vector.wait_ge(wsem, wval)
                select4(nc, tmp_pool, g, mk,
                        fsb[:, :nn, 18 + 6 * k:24 + 6 * k])
                if k == 2:
                    # chunk done: row i = base + 128*col + p (used cols only)
                    nc.scalar.dma_start(
                        fpk_d[base:base + ni, 0:36].rearrange(
                            "(c p) f -> p c f", p=P),
                        fsb[:, :nn, 0:36])
                    # guard chain: tiny read (waits write completion via
                    # Tile RAW), then bump fpk_sem -> chunk ci published
                    nc.scalar.dma_start(guard[0:1, 0:1],
                                        fpk_d[base:base + 1, 0:1])
                    nc.scalar.sem_inc(fpk_sem, 1)

            nc.gpsimd.collective_compute(
                "AllGather", mybir.AluOpType.bypass, replica_groups=groups,
                ins=[vnb.opt()], outs=[vn_tab.opt()])
            p4_calls = [(ci, k) for ci in range(len(p4sz)) for k in range(3)]
            pend = []
            for n_call, (ci, k) in enumerate(p4_calls):
                ni = p4sz[ci]
                it = idx_pool.tile([P, NI // 16], I16)
                nc.sync.dma_start(it[:, : ni // 16], p4i[k, ci, :, : ni // 16])
                g, q = gather_prep(gath_pool, vn_tab[:, :],
                                   it[:, : ni // 16], ni, "g1")
                pend.append((g, q, ci, k, ni))
                if len(pend) == NQ or n_call == len(p4_calls) - 1:
                    if not ag_emitted:
                        # order all triggers after the collective (the
                        # deferred table dep is not wired for collectives)
                        nc.gpsimd.wait_ge(ag_sem, 1)
                        ag_emitted = True
                    for (_, q2, _, _, _) in pend:
                        fire(q2)
                    for (g2, _, ci2, k2, ni2) in pend:
                        p4_consume(g2, ci2, k2, ni2)
                    pend = []

            # ---------------- P5: pixel phase ----------------

            # fpk chunk index needed for a given exclusive row bound
            cum = []
            tot = 0
            for ni in p4sz:
                tot += ni
                cum.append(tot)

            def chunk_of(row_excl):
                for idx_c, t in enumerate(cum):
                    if row_excl <= t:
                        return idx_c
                return len(cum) - 1

            pend5 = []
            for ci, (b, ni, d0, r0, r1) in enumerate(p5_plan):
                it = idx_pool.tile([P, NI5 // 16], I16, tag="it5")
                nc.sync.dma_start(it[:, : ni // 16], p5i[ci, :, : ni // 16])
                g, q = gather_prep(gath5_pool, fpk_d[r0:r1],
                                   it[:, : ni // 16], ni, "g5", cap=NI5)
                pend5.append((g, q, b, ni, d0, r1))
                if len(pend5) == NQ or ci == len(p5_plan) - 1:
                    need = max(chunk_of(e[-1]) for e in pend5) + 1
                    nc.gpsimd.wait_ge(fpk_sem, need)
                    for (_, q2, _, _, _, _) in pend5:
                        fire(q2)
                    for (g2, _, b2, ni2, d02, _) in pend5:
                        nn = ni2 // P
                        dcs = slice(d02 // P, (d02 + ni2) // P)
                        bw = px_pool.tile([P, NI5 // P, KPX, 36], F32,
                                          tag="bw", name="bw")
                        nc.sync.dma_start(bw[:, :nn], bw_in[:, dcs])
                        _pixel_math(nc, px_pool, g2, bw, nn, b2,
                                    o_pos[:, dcs], o_nrm[:, dcs],
                                    o_scr[:, dcs], o_dep[:, dcs])
                    pend5 = []

    nc.compile()
    return nc


def _patch_prep_sems(nc):
    """Bake Tile's DMASW lane sem into each PREPARE_ONLY gather descriptor.

    Tile schedules gen_mode==1 SWDGE preps on a DMASW proc lane and makes
    all data consumers / WAR successors wait on that lane's sem at the
    prep's cumulative tick, but the descriptor's completion sem slot
    (OnUpdate[0]) keeps the user sem= -- the lane sem then never
    increments and every one of those waits deadlocks. Preps on a lane all
    map to the same SWDGE queue (both rotate with emission order, 8 lanes /
    4 queues), and a queue completes its ring in order, so pointing
    OnUpdate[0] at the lane sem gives exactly the cumulative +16-per-DMA
    semantics the waits expect."""
    from concourse.tile_sem_assignment import PROC_NAME_TO_IDX
    idx_to_name = {v: k for k, v in PROC_NAME_TO_IDX.items()}
    sem_ids = {}
    insts = []
    for fn in nc.m.functions:
        for bb in fn.blocks:
            for ins in bb.instructions:
                si = ins.sync_info
                if si is None:
                    continue
                for ent in list(si.on_update) + list(si.on_wait):
                    if getattr(ent, "sync_type", None) == "semaphore"                             and ent.ant_name and "_" in ent.ant_name:
                        sem_ids.setdefault(
                            ent.ant_name.rsplit("_", 1)[0], ent.id)
                if type(ins).__name__ == "InstDMAGatherAnt"                         and getattr(ins, "gen_mode", 0) == 1:
                    insts.append(ins)
    for ins in insts:
        lane = idx_to_name[ins.bass_scheduled_proc]
        assert lane.startswith("DMASW"), lane
        assert lane in sem_ids, (lane, sorted(sem_ids))
        u0 = ins.sync_info.on_update[0]
        assert u0.update_value == 16, u0
        u0.id = sem_ids[lane]


def _pixel_math(nc, px_pool, g, bw, nn, b, d_pos, d_nrm, d_scr, d_dep):
    """Blend + project + normalize + depth for one gathered pixel call.

    g: [P, nn, 64] fpk rows; each desc serves KPX pixels (stride-0 reads).
    b is static for the call. Per-pixel payload bw [P, nn, KPX, 36] =
    [bary(0:3) Mrows-tiled-x3-verts(3:30) t(30:33) origin(33:36)].
    fpk row: [pos_b0(9: v0xyz v1xyz v2xyz) pos_b1(9) | vn(18 interleaved)].
    The projection q_i(k) = sum_j M_ij p_j(k) is one 9-wide mul against the
    host-tiled M row plus one innermost-3 reduce via an aliased
    [3*KPX, 3] view of the product tile.
    """
    sh = [P, nn, KPX, 3]
    bwa = bw[:, :nn, :, 0:3]
    po, no = 9 * b, 18 + 3 * b

    def grd_pos(j):
        # pos comp j of 3 verts (stride 3), broadcast over KPX
        return g[:, :nn, po + j:po + j + 7:3].unsqueeze(2).to_broadcast(sh)

    def grd_vn(j):
        # vn comp j of 3 verts (stride 6), broadcast over KPX
        return g[:, :nn, no + j:no + j + 13:6].unsqueeze(2).to_broadcast(sh)

    def cw(i, d=3):
        return bw[:, :nn, :, 3 + i:4 + i].to_broadcast([P, nn, KPX, d])

    def tl(tag, d=3, bufs=1):
        t = px_pool.tile([P, NI5 // P, KPX, d], F32, tag=tag, name=tag,
                         bufs=bufs)
        return t[:, :nn]

    red = mybir.AxisListType.X
    t3 = tl("t3")
    bpos = tl("bpos", bufs=2)
    bnrm = tl("bnrm")
    for j in range(3):
        _mul(nc, t3, grd_pos(j), bwa)
        nc.vector.tensor_reduce(out=bpos[:, :, :, j:j + 1], in_=t3, axis=red,
                                op=mybir.AluOpType.add)
        _mul(nc, t3, grd_vn(j), bwa)
        nc.vector.tensor_reduce(out=bnrm[:, :, :, j:j + 1], in_=t3, axis=red,
                                op=mybir.AluOpType.add)

    # screen: q_i(k) = sum_j M[i,j] * p_j(k) + t_i per vertex, then blend
    t9t = px_pool.tile([P, NI5 // P, KPX * 3, 3], F32, tag="t9", name="t9",
                       bufs=1)
    t9r = t9t[:, :nn]                                  # [P,nn,3KPX,3]
    t9m = t9r.rearrange("p n (x j) c -> p n x (j c)", j=3)  # [P,nn,KPX,9]
    gpos9 = g[:, :nn, po:po + 9].unsqueeze(2).to_broadcast([P, nn, KPX, 9])
    q = [tl(f"q{i}") for i in range(3)]
    for i in range(3):
        _mul(nc, t9m, gpos9, bw[:, :nn, :, 3 + 9 * i:12 + 9 * i])
        qv = q[i].rearrange("p n x c -> p n (x c)").unsqueeze(3)
        nc.vector.tensor_reduce(out=qv, in_=t9r, axis=red,
                                op=mybir.AluOpType.add)
        _add(nc, q[i], q[i], cw(27 + i))
    rz = tl("rz")
    nc.vector.reciprocal_approx_fast(rz.opt(), q[2].opt())
    _mul(nc, rz, rz, bwa)                        # rz := bary/qz
    scr = tl("scr", bufs=2)
    for i in range(2):
        _mul(nc, t3, q[i], rz)
        nc.vector.tensor_reduce(out=scr[:, :, :, i:i + 1], in_=t3, axis=red,
                                op=mybir.AluOpType.add)
    _mul(nc, t3, q[2], bwa)
    nc.vector.tensor_reduce(out=scr[:, :, :, 2:3], in_=t3, axis=red,
                            op=mybir.AluOpType.add)

    # unit normal: bnrm / sqrt(|bnrm|^2); the square+reduce runs on the
    # gpsimd tensor ALU (vector is saturated in the pixel phase)
    s1 = tl("s1", 1)
    tg = tl("tg")
    nc.gpsimd.tensor_tensor(out=tg, in0=bnrm, in1=bnrm,
                            op=mybir.AluOpType.mult)
    nc.vector.tensor_reduce(out=s1, in_=tg, axis=red,
                            op=mybir.AluOpType.add)
    sq = tl("sq", 1)
    nc.scalar.sqrt(sq, s1)
    nc.vector.reciprocal_approx_fast(s1.opt(), sq.opt())
    nrm = tl("nrm", bufs=2)
    _mul(nc, nrm, bnrm, s1[:, :, :, 0:1].to_broadcast(sh))

    # depth = |origin - bpos| entirely on gpsimd + scalar
    dv = tl("dv")
    nc.gpsimd.tensor_tensor(out=dv, in0=bpos, in1=bw[:, :nn, :, 33:36],
                            op=mybir.AluOpType.subtract)
    nc.gpsimd.tensor_tensor(out=tg, in0=dv, in1=dv,
                            op=mybir.AluOpType.mult)
    s1d = tl("s1d", 1)
    nc.vector.tensor_reduce(out=s1d, in_=tg, axis=red,
                            op=mybir.AluOpType.add)
    dep = tl("dep", 1, bufs=2)
    nc.scalar.sqrt(dep, s1d)

    nc.scalar.dma_start(d_pos, bpos)
    nc.scalar.dma_start(d_nrm, nrm)
    nc.scalar.dma_start(d_scr, scr)
    nc.scalar.dma_start(d_dep, dep[:, :, :, 0])


_PROG_CACHE = {}


def _get_program(cfg, meta):
    key = (tuple(meta["p5_plan"]), meta["nd_cols"])
    if key not in _PROG_CACHE:
        _PROG_CACHE[key] = build_program(cfg, meta)
    return _PROG_CACHE[key]


def run(cfg, inputs, trace=False):
    percore, meta = host_prep(cfg, inputs)
    nc = _get_program(cfg, meta)
    res = run_bass_kernel_spmd(nc, percore, core_ids=list(range(8)),
                               trace=trace)
    return res, meta


def assemble(cfg, inputs, res, meta):
    c = cfg
    fid = np.asarray(inputs["face_id"])
    out = np.zeros((5, c.NBC, c.H * c.W, 3), np.float32)
    nd_cols = meta["nd_cols"]
    for core in range(8):
        r = res.results[core]
        # slot (desc d, kk) -> grid position (p=d%128, col=d//128, kk)
        pos = r["o_pos"].reshape(P, nd_cols, KPX, 3)
        nrm = r["o_nrm"].reshape(P, nd_cols, KPX, 3)
        scr = r["o_scr"].reshape(P, nd_cols, KPX, 3)
        dep = r["o_dep"].reshape(P, nd_cols, KPX)
        for (sel, bcv, didx, kk) in meta["px"][core]:
            if not len(sel):
                continue
            p, col = didx % P, didx // P
            out[1, bcv, sel] = pos[p, col, kk]
            out[2, bcv, sel] = nrm[p, col, kk]
            out[4, bcv, sel] = scr[p, col, kk]
            out[3, bcv, sel] = dep[p, col, kk][:, None]
    out = out.reshape(5, c.B, c.C, c.H, c.W, 3)
    m = (np.asarray(fid) >= 0).astype(np.float32)
    out[0] = np.broadcast_to(m[:, :, :, :, None], out[0].shape)
    return out.astype(np.float32)


def kernel(**inputs):
    cfg = Cfg()
    res, meta = run(cfg, inputs)
    return assemble(cfg, inputs, res, meta)


# revision 51
# speedup vs baseline: 1.3410x; 1.3410x over previous
"""Trainium2 Bass kernel for nn_CudaRendererGpu (differentiable-renderer forward).

Strategy (8 NeuronCores, SPMD), v16:
  Faces and vertices are sharded 8 ways (core c owns faces [25000c, 25000(c+1))
  and verts [12500c, 12500(c+1))). All per-vertex INPUT data (positions of the
  corner vertices of each vertex's 8 adjacent faces) is expanded on the HOST
  into contiguous per-core streams, so vertex normals are computed with ZERO
  gather descriptors (gather descriptor generation on the GPSIMD Q7 cores,
  ~8ns/idx on 2-of-8 cores per SWDGE queue, is the machine bottleneck).
  Only two gather phases remain, both split into 2048-idx calls round-robined
  over all 4 SWDGE queues with deep buffering so four descriptor generators
  run concurrently:
    P4: vn at face corners from a 4-packed all-gathered vn table; the 4-way
        sub-slot select is one mul over a [6f,4s] strided view + one reduce.
    P5: pixel phase over the packed face table fpk (pos streamed from host,
        vn from P4), pixels sorted by (b, face id), KPX=4 pixels per
        256B descriptor; b0/b1 calls interleaved so each call's fpk row
        range rises monotonically (pipelines against P4 chunk completion).
  Pixel math fuses the 3x3 projection into one 9-wide mul against a
  host-tiled M-row + an innermost-3 reduce through an aliased [3*KPX,3]
  view. Input loads ride the sync queue; result writes ride the scalar
  queue so they never head-of-line block input prefetch.
  Host does index composition/permutation only; all float math and all
  device-computed-table data movement happens on device.
"""

import numpy as np

import concourse.bass as bass
import concourse.mybir as mybir
import concourse.tile as tile
from concourse import bacc
from concourse.bass_utils import run_bass_kernel_spmd

F32 = mybir.dt.float32
I16 = mybir.dt.int16
P = 128
NI = 2048                # idxs per dma_gather call (P4)
NI5 = 4096               # idxs per pixel-phase gather call
KPX = 4                  # pixels packed per pixel-phase descriptor
ACH = 10                 # phase-A chunk columns (10 cols = 1280 verts)


def _ceil128(x):
    return (x + 127) // 128 * 128


class Cfg:
    def __init__(self, B=2, C=4, H=512, W=512, V=100000, F=200000, A=8):
        self.B, self.C, self.H, self.W, self.V, self.F, self.A = B, C, H, W, V, F, A
        self.NBC = B * C
        self.FSHV = F // 8            # valid faces per shard
        self.VSHV = V // 8
        self.FSH = 25600
        self.VSH = 12800
        self.FCOLS = self.FSH // P    # 200
        self.VCOLS = self.VSH // P    # 100
        self.VN_ROWS_L = P * (self.VCOLS // 4)   # 3200 vn rows per core
        self.VN_ROWS = 8 * self.VN_ROWS_L        # 25600 global (int16-safe)


def _call_sizes(total, ni=NI):
    out = []
    left = total
    while left > 0:
        c = min(ni, left)
        out.append(c)
        left -= c
    return out


def _call_sizes5(total):
    return _call_sizes(total, NI5)


def _wrap16(idx):
    """[N] (N%16==0) int array -> dma_gather idx layout [128, N//16]."""
    w = idx.reshape(-1, 16).T.astype(np.int16)
    return np.tile(w, (8, 1))


def _pack_calls(idx16, sizes, ni=NI):
    wi = np.zeros((len(sizes), P, ni // 16), np.int16)
    base = 0
    for i, n in enumerate(sizes):
        wi[i, :, : n // 16] = _wrap16(idx16[base:base + n])
        base += n
    return wi


def _grid_masks(sub, vals, cols, nsub):
    """sub [N], vals [N] -> m [P, cols, nsub]; grid slot i=(p=i%128, c=i//128)."""
    m = np.zeros((P, cols, nsub), np.float32)
    sg = sub.reshape(cols, P).T          # [P, cols]
    vg = vals.reshape(cols, P).T
    for s in range(nsub):
        m[:, :, s] = np.where(sg == s, vg, 0.0)
    return m


def _vn_row_sub(cfg, v):
    """global vertex id -> (vn_tab row, sub) for the 4-packed vn table."""
    c = cfg
    sh, j = np.divmod(v, c.VSHV)
    p, col = j % P, j // P
    return sh * c.VN_ROWS_L + p * (c.VCOLS // 4) + (col >> 2), col & 3


def host_prep(cfg, inputs):
    c = cfg
    faces = np.asarray(inputs["faces"]).astype(np.int64)
    vaf = np.asarray(inputs["vert_adj_faces"]).astype(np.int64)
    vaw = np.asarray(inputs["vert_adj_weights"]).astype(np.float32)
    vpos = np.asarray(inputs["vertex_pos"]).astype(np.float32)
    intr = np.asarray(inputs["intrinsics"]).astype(np.float32)
    extr = np.asarray(inputs["extrinsics"]).astype(np.float32)
    iext = np.asarray(inputs["inverse_extrinsics"]).astype(np.float32)
    fid = np.asarray(inputs["face_id"]).astype(np.int64)
    bary = np.asarray(inputs["barycentrics"]).astype(np.float32)

    # ---- per-bc projection constants, streamed per pixel slot ----
    # [M rows, each tiled x3 over verts (27) | t (3) | origin (3)]
    cst33 = np.zeros((c.NBC, 33), np.float32)
    for b in range(c.B):
        for cam in range(c.C):
            M = intr[b, cam] @ extr[b, cam][:, :3]
            t = intr[b, cam] @ extr[b, cam][:, 3]
            col = iext[b, cam][:, 3]
            orig = (col / col[3])[:3]
            for i in range(3):
                cst33[b * c.C + cam, 9 * i:9 * i + 9] = np.tile(M[i], 3)
            cst33[b * c.C + cam, 27:30] = t
            cst33[b * c.C + cam, 30:33] = orig

    p4sz = _call_sizes(c.FSH)

    # ---- expanded corner-position streams (pure permutations of inputs) ----
    # phase A layout: [v0b0 v0b1 v1b0 v1b1 v2b0 v2b1] (vert-major, b inner)
    fc18 = np.empty((c.F, 18), np.float32)
    for k in range(3):
        fc18[:, 6 * k:6 * k + 3] = vpos[0, faces[:, k]]
        fc18[:, 6 * k + 3:6 * k + 6] = vpos[1, faces[:, k]]
    # fpk pos layout: [pos_b0 (v0xyz v1xyz v2xyz), pos_b1 (9)]
    fp18 = np.concatenate([vpos[0, faces].reshape(c.F, 9),
                           vpos[1, faces].reshape(c.F, 9)], 1)

    # ---- pixel phase: per-core routing, one face-sorted stream per core ----
    fimg = fid.reshape(c.NBC, -1)                      # [8, H*W]
    grp = []                                           # per (core, b)
    ndesc = np.zeros((8, c.B), np.int64)
    for core in range(8):
        ent = []
        for b in range(c.B):
            sels, js, bcs = [], [], []
            for cam in range(c.C):
                bc = b * c.C + cam
                f = fimg[bc]
                sel = np.where((f >= core * c.FSHV)
                               & (f < (core + 1) * c.FSHV))[0]
                sels.append(sel)
                js.append(f[sel] - core * c.FSHV)
                bcs.append(np.full(len(sel), bc, np.int64))
            sel = np.concatenate(sels)
            j = np.concatenate(js)
            bcv = np.concatenate(bcs)
            order = np.argsort(j, kind="stable")
            sel, j, bcv = sel[order], j[order], bcv[order]
            brk = np.nonzero(np.diff(j))[0] + 1
            starts = np.concatenate([[0], brk]).astype(np.int64)
            lens = np.diff(np.concatenate([starts, [len(j)]]))
            dper = -(-lens // KPX)
            ndesc[core, b] = dper.sum()
            ent.append((sel, j, bcv, starts, lens, dper))
        grp.append(ent)
    caps = [max(NI5, int(_ceil128(ndesc[:, b].max()))) for b in range(c.B)]
    seg_base = [0, caps[0]]
    ndesc_tot = caps[0] + caps[1]
    nd_cols = ndesc_tot // P
    p5szs = []                                         # (b, ni, d0)
    per_b = []
    for b in range(c.B):
        off = seg_base[b]
        ent = []
        for ni in _call_sizes5(caps[b]):
            ent.append((b, ni, off))
            off += ni
        per_b.append(ent)
    # interleave b0/b1 calls so each call's fpk row range (and thus its
    # dependency on P4 chunks) increases monotonically with emission order
    for i in range(max(len(e) for e in per_b)):
        for ent in per_b:
            if i < len(ent):
                p5szs.append(ent[i])

    percore = []
    meta_px = []
    for core in range(8):
        im = {}
        f0 = core * c.FSHV
        v0 = core * c.VSHV
        # ---- phase A: expanded adjacency corner positions + weights ----
        av = np.zeros((c.VSH, c.A), np.int64)
        wv = np.zeros((c.VSH, c.A), np.float32)
        av[:c.VSHV] = vaf[v0:v0 + c.VSHV]
        wv[:c.VSHV] = vaw[v0:v0 + c.VSHV]
        ex = np.zeros((c.VSH, 18, c.A), np.float32)
        ex[:c.VSHV] = fc18[av[:c.VSHV]].transpose(0, 2, 1)
        # grid slot i=(p=i%128, c=i//128): [VSH,18,A] -> [P, VCOLS, 18, A]
        im["exp"] = (ex.reshape(c.VCOLS, P, 18, c.A)
                     .transpose(1, 0, 2, 3).copy())
        im["wts"] = (wv.reshape(c.VCOLS, P, c.A)
                     .transpose(1, 0, 2).copy())
        # ---- fpos: per-face corner positions in fpk layout (cols 0:18) ----
        fp = np.zeros((c.FSH, 18), np.float32)
        fp[:c.FSHV] = fp18[f0:f0 + c.FSHV]
        im["fpos"] = fp.reshape(c.FCOLS, P, 18).transpose(1, 0, 2).copy()
        # ---- P4: vn at corners (4-packed table, select4) ----
        fk = np.zeros((c.FSH, 3), np.int64)
        fk[:c.FSHV] = faces[f0:f0 + c.FSHV]
        p4i, p4m = [], []
        for k in range(3):
            row, sub = _vn_row_sub(c, fk[:, k])
            p4i.append(_pack_calls(row.astype(np.int16), p4sz))
            p4m.append(_grid_masks(sub, np.ones(c.FSH, np.float32),
                                   c.FCOLS, 4))
        im["p4i"] = np.stack(p4i)
        im["p4m"] = np.stack(p4m)
        # ---- P5: pixel descriptors + per-pixel payload ----
        drows = np.zeros(ndesc_tot, np.int64)
        bw = np.zeros((ndesc_tot, KPX, 36), np.float32)
        bw[:, :, 0] = 1.0                  # pad slots: bary=[1,0,0] ...
        bw[:, :, 32] = 1.0                 # ... and t2=1 keep all math finite
        px_meta = []
        for b in range(c.B):
            sel, j, bcv, starts, lens, dper = grp[core][b]
            base = seg_base[b]
            nvalid = int(dper.sum())
            dr = np.repeat(j[starts], dper)
            drows[base:base + nvalid] = dr
            drows[base + nvalid:base + caps[b]] = dr[-1] if nvalid else 0
            pos_in_run = np.arange(len(j)) - np.repeat(starts, lens)
            dbase = np.concatenate([[0], np.cumsum(dper)])[:-1]
            didx = base + np.repeat(dbase, lens) + pos_in_run // KPX
            kk = pos_in_run % KPX
            bw[didx, kk, 0:3] = bary.reshape(c.NBC, -1, 3)[bcv, sel]
            bw[didx, kk, 3:36] = cst33[bcv]
            px_meta.append((sel, bcv, didx, kk))
        im["drows"] = drows                # int64, converted per-call below
        im["bw"] = bw.reshape(nd_cols, P, KPX, 36).transpose(1, 0, 2, 3).copy()
        percore.append(im)
        meta_px.append(px_meta)

    # per-call fpk row ranges unioned across cores, then relative idx streams
    p5_plan = []
    for b, ni, off in p5szs:
        r0 = min(int(im["drows"][off:off + ni].min()) for im in percore)
        r1 = max(int(im["drows"][off:off + ni].max()) for im in percore) + 1
        assert r1 - r0 < 32768
        p5_plan.append((b, ni, off, r0, r1))
    for im in percore:
        p5i = np.zeros((len(p5_plan), P, NI5 // 16), np.int16)
        for ci, (b, ni, d0, r0, r1) in enumerate(p5_plan):
            p5i[ci, :, : ni // 16] = _wrap16(im["drows"][d0:d0 + ni] - r0)
        im["p5i"] = p5i
        del im["drows"]

    meta = dict(p4sz=p4sz, p5_plan=p5_plan, nd_cols=nd_cols, px=meta_px)
    return percore, meta


def _mul(nc, out, a, b):
    nc.vector.tensor_tensor(out=out, in0=a, in1=b, op=mybir.AluOpType.mult)


def _add(nc, out, a, b):
    nc.vector.tensor_tensor(out=out, in0=a, in1=b, op=mybir.AluOpType.add)


def _sub(nc, out, a, b):
    nc.vector.tensor_tensor(out=out, in0=a, in1=b, op=mybir.AluOpType.subtract)


def build_program(cfg, meta, num_swdge_queues=4):
    c = cfg
    p4sz = meta["p4sz"]
    p5_plan = meta["p5_plan"]
    nd_cols = meta["nd_cols"]
    NQ = num_swdge_queues

    nc = bacc.Bacc("TRN2", target_bir_lowering=False, debug=False,
                   num_devices=8, num_swdge_queues=NQ,
                   dynamic_dma_scratch_size=32768)

    def din(name, shape, dt=F32):
        return nc.dram_tensor(name, list(shape), dt, kind="ExternalInput").ap()

    exp_in = din("exp", [P, c.VCOLS, 18, c.A])
    wts_in = din("wts", [P, c.VCOLS, c.A])
    fpos_in = din("fpos", [P, c.FCOLS, 18])
    p4i = din("p4i", [3, len(p4sz), P, NI // 16], I16)
    p4m = din("p4m", [3, P, c.FCOLS, 4])
    p5i = din("p5i", [len(p5_plan), P, NI5 // 16], I16)
    bw_in = din("bw", [P, nd_cols, KPX, 36])

    o_pos = nc.dram_tensor("o_pos", [P, nd_cols, KPX, 3], F32,
                           kind="ExternalOutput").ap()
    o_nrm = nc.dram_tensor("o_nrm", [P, nd_cols, KPX, 3], F32,
                           kind="ExternalOutput").ap()
    o_scr = nc.dram_tensor("o_scr", [P, nd_cols, KPX, 3], F32,
                           kind="ExternalOutput").ap()
    o_dep = nc.dram_tensor("o_dep", [P, nd_cols, KPX], F32,
                           kind="ExternalOutput").ap()

    groups = [list(range(8))]
    qctr = [0]

    def gather_prep(pool, tab_ap, idx_tile, ni, tag, cap=NI):
        g = pool.tile([P, cap // P, 64], F32, tag=tag, name=tag)
        q = qctr[0] % NQ
        nc.gpsimd.dma_gather(
            out_ap=g[:, : ni // P, :], in_ap=tab_ap, idxs_ap=idx_tile,
            num_idxs=ni, num_idxs_reg=ni, elem_size=64,
            single_packet=False, queue_num=q)
        qctr[0] += 1
        return g, q

    def fire(q):
        pass

    def select4(nc, tmp_pool, g, msk, out):
        """out[:, :, 0:6] = sum_s msk[..., s] * g[:, :, 16s:16s+6].

        One mul over a [6f, 4s] strided view (f stride 1, s stride 16) and
        one innermost-4 reduce."""
        nn = out.shape[1]
        gv = g[:, :nn, :].rearrange("p n (s f) -> p n f s", s=4)[:, :, 0:6, :]
        mb = msk[:, :nn, :].unsqueeze(2).to_broadcast([P, nn, 6, 4])
        t = tmp_pool.tile([P, NI // P, 6, 4], F32, tag="selt", bufs=2,
                          name="selt")[:, :nn]
        _mul(nc, t, gv, mb)
        nc.vector.tensor_reduce(out=out.unsqueeze(3), in_=t,
                                axis=mybir.AxisListType.X,
                                op=mybir.AluOpType.add)

    with tile.TileContext(nc) as tc:
        with tc.tile_pool(name="dram", bufs=1, space="DRAM") as dram, \
             tc.tile_pool(name="expp", bufs=2) as exp_pool, \
             tc.tile_pool(name="atmp", bufs=1) as atmp_pool, \
             tc.tile_pool(name="acc", bufs=1) as acc_pool, \
             tc.tile_pool(name="gath", bufs=6) as gath_pool, \
             tc.tile_pool(name="gath5", bufs=4) as gath5_pool, \
             tc.tile_pool(name="idx", bufs=16) as idx_pool, \
             tc.tile_pool(name="msk", bufs=10) as msk_pool, \
             tc.tile_pool(name="tmp", bufs=1) as tmp_pool, \
             tc.tile_pool(name="fpk", bufs=2) as fpk_pool, \
             tc.tile_pool(name="px", bufs=2) as px_pool:

            vn_tab = dram.tile([c.VN_ROWS, 64], F32, tag="vn_tab")
            vnb = dram.tile([P, c.VCOLS // 4, 64], F32, tag="vnb")
            fpk_d = dram.tile([c.FSH, 64], F32, tag="fpk")

            # ---------------- A: vertex normals, no gathers ----------------
            vna = acc_pool.tile([P, c.VCOLS, 6], F32, tag="vna", name="vna")
            for c0 in range(0, c.VCOLS, ACH):
                cs = slice(c0, c0 + ACH)
                ex = exp_pool.tile([P, ACH, 18, c.A], F32, tag="ex", name="ex")
                nc.sync.dma_start(ex[:], exp_in[:, cs, :, :])
                w = exp_pool.tile([P, ACH, c.A], F32, tag="w", name="w")
                nc.sync.dma_start(w[:], wts_in[:, cs, :])
                e1 = atmp_pool.tile([P, ACH, 6, c.A], F32, tag="e1", name="e1")
                e2 = atmp_pool.tile([P, ACH, 6, c.A], F32, tag="e2", name="e2")
                _sub(nc, e1[:], ex[:, :, 6:12, :], ex[:, :, 0:6, :])
                _sub(nc, e2[:], ex[:, :, 12:18, :], ex[:, :, 0:6, :])
                # scalar-engine sigma/tau component shuffles turn the 18
                # per-component cross ops into 3 wide vector ops:
                # cr = e1s*e2t - e1t*e2s  (per b: s=(y,z,x), t=(z,x,y))
                perms = {}
                for nm, srct in (("e1s", e1), ("e1t", e1),
                                 ("e2s", e2), ("e2t", e2)):
                    dstt = atmp_pool.tile([P, ACH, 6, c.A], F32, tag=nm,
                                          name=nm)
                    sig = nm[2] == "s"
                    for b in range(2):
                        if sig:   # [1,2,0]
                            nc.scalar.copy(out=dstt[:, :, 3*b:3*b+2, :],
                                           in_=srct[:, :, 3*b+1:3*b+3, :])
                            nc.scalar.copy(out=dstt[:, :, 3*b+2:3*b+3, :],
                                           in_=srct[:, :, 3*b:3*b+1, :])
                        else:     # [2,0,1]
                            nc.scalar.copy(out=dstt[:, :, 3*b:3*b+1, :],
                                           in_=srct[:, :, 3*b+2:3*b+3, :])
                            nc.scalar.copy(out=dstt[:, :, 3*b+1:3*b+3, :],
                                           in_=srct[:, :, 3*b:3*b+2, :])
                    perms[nm] = dstt
                cr = perms["e1s"]
                _mul(nc, cr[:], cr[:], perms["e2t"][:])
                _mul(nc, perms["e1t"][:], perms["e1t"][:], perms["e2s"][:])
                _sub(nc, cr[:], cr[:], perms["e1t"][:])
                wb = w[:, :, :].unsqueeze(2).to_broadcast([P, ACH, 6, c.A])
                _mul(nc, cr[:], cr[:], wb)
                nc.vector.tensor_reduce(out=vna[:, cs, :].unsqueeze(3),
                                        in_=cr[:], axis=mybir.AxisListType.X,
                                        op=mybir.AluOpType.add)

            # repack into 4-packed rows (AllGather emitted below, after the
            # first round of P4 preps so their desc-gen is not blocked
            # behind the collective in the gpsimd stream)
            vnb_s = acc_pool.tile([P, c.VCOLS // 4, 64], F32, tag="vnb_s",
                                  name="vnb_s")
            nc.vector.memset(vnb_s[:], 0.0)
            for s in range(4):
                nc.scalar.copy(out=vnb_s[:, :, 16 * s:16 * s + 6],
                               in_=vna[:, s::4, :])
            nc.scalar.dma_start(vnb[:], vnb_s[:])

            # ---------------- P4: fpk assembly ----------------
            # fpk row: [pos(18): v0b0 v0b1 v1b0 v1b1 v2b0 v2b1 | vn(18) | pad]
            # Emission in rounds of NQ: preps (desc-gen, no table dep), then
            # triggers (gated on vn_tab), then select4 consumers. Input loads
            # go on the sync queue; result writes go on the scalar queue so
            # they never head-of-line block input prefetch.
            fsb_tiles = {}


            def p4_consume(g, ci, k, ni):
                nn = ni // P
                base = sum(p4sz[:ci])
                cs = slice(base // P, (base + ni) // P)
                if ci not in fsb_tiles:
                    fsb_tiles[ci] = fpk_pool.tile([P, NI // P, 64], F32,
                                                  tag="fsb", name="fsb")
                    nc.sync.dma_start(fsb_tiles[ci][:, :nn, 0:18],
                                      fpos_in[:, cs, :])
                fsb = fsb_tiles[ci]
                mk = msk_pool.tile([P, NI // P, 4], F32)
                nc.sync.dma_start(mk[:, :nn, :], p4m[k, :, cs, :])
                # Tile's DMASW wait on a PREPARE_ONLY gather fires at
                # desc-gen, not DMA completion -- gate on the baked
                # per-queue sem (cumulative, in ring order).
                nc.vector.wait_ge(wsem, wval)
                select4(nc, tmp_pool, g, mk,
                        fsb[:, :nn, 18 + 6 * k:24 + 6 * k])
                if k == 2:
                    # chunk done: row i = base + 128*col + p (used cols only)
                    nc.scalar.dma_start(
                        fpk_d[base:base + ni, 0:36].rearrange(
                            "(c p) f -> p c f", p=P),
                        fsb[:, :nn, 0:36])
                    # guard chain: tiny read (waits write completion via
                    # Tile RAW), then bump fpk_sem -> chunk ci published
                    nc.scalar.dma_start(guard[0:1, 0:1],
                                        fpk_d[base:base + 1, 0:1])
                    nc.scalar.sem_inc(fpk_sem, 1)

            nc.gpsimd.collective_compute(
                "AllGather", mybir.AluOpType.bypass, replica_groups=groups,
                ins=[vnb.opt()], outs=[vn_tab.opt()])
            p4_calls = [(ci, k) for ci in range(len(p4sz)) for k in range(3)]
            pend = []
            for n_call, (ci, k) in enumerate(p4_calls):
                ni = p4sz[ci]
                it = idx_pool.tile([P, NI // 16], I16)
                nc.sync.dma_start(it[:, : ni // 16], p4i[k, ci, :, : ni // 16])
                g, q = gather_prep(gath_pool, vn_tab[:, :],
                                   it[:, : ni // 16], ni, "g1")
                pend.append((g, q, ci, k, ni))
                if len(pend) == NQ or n_call == len(p4_calls) - 1:
                    if not ag_emitted:
                        # order all triggers after the collective (the
                        # deferred table dep is not wired for collectives)
                        nc.gpsimd.wait_ge(ag_sem, 1)
                        ag_emitted = True
                    for (_, q2, _, _, _) in pend:
                        fire(q2)
                    for (g2, _, ci2, k2, ni2) in pend:
                        p4_consume(g2, ci2, k2, ni2)
                    pend = []

            # ---------------- P5: pixel phase ----------------

            # fpk chunk index needed for a given exclusive row bound
            cum = []
            tot = 0
            for ni in p4sz:
                tot += ni
                cum.append(tot)

            def chunk_of(row_excl):
                for idx_c, t in enumerate(cum):
                    if row_excl <= t:
                        return idx_c
                return len(cum) - 1

            pend5 = []
            for ci, (b, ni, d0, r0, r1) in enumerate(p5_plan):
                it = idx_pool.tile([P, NI5 // 16], I16, tag="it5")
                nc.sync.dma_start(it[:, : ni // 16], p5i[ci, :, : ni // 16])
                g, q = gather_prep(gath5_pool, fpk_d[r0:r1],
                                   it[:, : ni // 16], ni, "g5", cap=NI5)
                pend5.append((g, q, b, ni, d0, r1))
                if len(pend5) == NQ or ci == len(p5_plan) - 1:
                    need = max(chunk_of(e[-1]) for e in pend5) + 1
                    nc.gpsimd.wait_ge(fpk_sem, need)
                    for (_, q2, _, _, _, _) in pend5:
                        fire(q2)
                    for (g2, _, b2, ni2, d02, _) in pend5:
                        nn = ni2 // P
                        dcs = slice(d02 // P, (d02 + ni2) // P)
                        bw = px_pool.tile([P, NI5 // P, KPX, 36], F32,
                                          tag="bw", name="bw")
                        nc.sync.dma_start(bw[:, :nn], bw_in[:, dcs])
                        _pixel_math(nc, px_pool, g2, bw, nn, b2,
                                    o_pos[:, dcs], o_nrm[:, dcs],
                                    o_scr[:, dcs], o_dep[:, dcs])
                    pend5 = []

    nc.compile()
    return nc


def _patch_prep_sems(nc):
    """Bake Tile's DMASW lane sem into each PREPARE_ONLY gather descriptor.

    Tile schedules gen_mode==1 SWDGE preps on a DMASW proc lane and makes
    all data consumers / WAR successors wait on that lane's sem at the
    prep's cumulative tick, but the descriptor's completion sem slot
    (OnUpdate[0]) keeps the user sem= -- the lane sem then never
    increments and every one of those waits deadlocks. Preps on a lane all
    map to the same SWDGE queue (both rotate with emission order, 8 lanes /
    4 queues), and a queue completes its ring in order, so pointing
    OnUpdate[0] at the lane sem gives exactly the cumulative +16-per-DMA
    semantics the waits expect."""
    from concourse.tile_sem_assignment import PROC_NAME_TO_IDX
    idx_to_name = {v: k for k, v in PROC_NAME_TO_IDX.items()}
    sem_ids = {}
    insts = []
    for fn in nc.m.functions:
        for bb in fn.blocks:
            for ins in bb.instructions:
                si = ins.sync_info
                if si is None:
                    continue
                for ent in list(si.on_update) + list(si.on_wait):
                    if getattr(ent, "sync_type", None) == "semaphore"                             and ent.ant_name and "_" in ent.ant_name:
                        sem_ids.setdefault(
                            ent.ant_name.rsplit("_", 1)[0], ent.id)
                if type(ins).__name__ == "InstDMAGatherAnt"                         and getattr(ins, "gen_mode", 0) == 1:
                    insts.append(ins)
    for ins in insts:
        lane = idx_to_name[ins.bass_scheduled_proc]
        assert lane.startswith("DMASW"), lane
        assert lane in sem_ids, (lane, sorted(sem_ids))
        u0 = ins.sync_info.on_update[0]
        assert u0.update_value == 16, u0
        u0.id = sem_ids[lane]


def _pixel_math(nc, px_pool, g, bw, nn, b, d_pos, d_nrm, d_scr, d_dep):
    """Blend + project + normalize + depth for one gathered pixel call.

    g: [P, nn, 64] fpk rows; each desc serves KPX pixels (stride-0 reads).
    b is static for the call. Per-pixel payload bw [P, nn, KPX, 36] =
    [bary(0:3) Mrows-tiled-x3-verts(3:30) t(30:33) origin(33:36)].
    fpk row: [pos_b0(9: v0xyz v1xyz v2xyz) pos_b1(9) | vn(18 interleaved)].
    The projection q_i(k) = sum_j M_ij p_j(k) is one 9-wide mul against the
    host-tiled M row plus one innermost-3 reduce via an aliased
    [3*KPX, 3] view of the product tile.
    """
    sh = [P, nn, KPX, 3]
    bwa = bw[:, :nn, :, 0:3]
    po, no = 9 * b, 18 + 3 * b

    def grd_pos(j):
        # pos comp j of 3 verts (stride 3), broadcast over KPX
        return g[:, :nn, po + j:po + j + 7:3].unsqueeze(2).to_broadcast(sh)

    def grd_vn(j):
        # vn comp j of 3 verts (stride 6), broadcast over KPX
        return g[:, :nn, no + j:no + j + 13:6].unsqueeze(2).to_broadcast(sh)

    def cw(i, d=3):
        return bw[:, :nn, :, 3 + i:4 + i].to_broadcast([P, nn, KPX, d])

    def tl(tag, d=3, bufs=1):
        t = px_pool.tile([P, NI5 // P, KPX, d], F32, tag=tag, name=tag,
                         bufs=bufs)
        return t[:, :nn]

    red = mybir.AxisListType.X
    t3 = tl("t3")
    bpos = tl("bpos", bufs=2)
    bnrm = tl("bnrm")
    for j in range(3):
        _mul(nc, t3, grd_pos(j), bwa)
        nc.vector.tensor_reduce(out=bpos[:, :, :, j:j + 1], in_=t3, axis=red,
                                op=mybir.AluOpType.add)
        _mul(nc, t3, grd_vn(j), bwa)
        nc.vector.tensor_reduce(out=bnrm[:, :, :, j:j + 1], in_=t3, axis=red,
                                op=mybir.AluOpType.add)

    # screen: q_i(k) = sum_j M[i,j] * p_j(k) + t_i per vertex, then blend
    t9t = px_pool.tile([P, NI5 // P, KPX * 3, 3], F32, tag="t9", name="t9",
                       bufs=1)
    t9r = t9t[:, :nn]                                  # [P,nn,3KPX,3]
    t9m = t9r.rearrange("p n (x j) c -> p n x (j c)", j=3)  # [P,nn,KPX,9]
    gpos9 = g[:, :nn, po:po + 9].unsqueeze(2).to_broadcast([P, nn, KPX, 9])
    q = [tl(f"q{i}") for i in range(3)]
    for i in range(3):
        _mul(nc, t9m, gpos9, bw[:, :nn, :, 3 + 9 * i:12 + 9 * i])
        qv = q[i].rearrange("p n x c -> p n (x c)").unsqueeze(3)
        nc.vector.tensor_reduce(out=qv, in_=t9r, axis=red,
                                op=mybir.AluOpType.add)
        _add(nc, q[i], q[i], cw(27 + i))
    rz = tl("rz")
    nc.vector.reciprocal_approx_fast(rz.opt(), q[2].opt())
    _mul(nc, rz, rz, bwa)                        # rz := bary/qz
    scr = tl("scr", bufs=2)
    for i in range(2):
        _mul(nc, t3, q[i], rz)
        nc.vector.tensor_reduce(out=scr[:, :, :, i:i + 1], in_=t3, axis=red,
                                op=mybir.AluOpType.add)
    _mul(nc, t3, q[2], bwa)
    nc.vector.tensor_reduce(out=scr[:, :, :, 2:3], in_=t3, axis=red,
                            op=mybir.AluOpType.add)

    # unit normal: bnrm / sqrt(|bnrm|^2)
    s1 = tl("s1", 1)
    _mul(nc, t3, bnrm, bnrm)
    nc.vector.tensor_reduce(out=s1, in_=t3, axis=red, op=mybir.AluOpType.add)
    sq = tl("sq", 1)
    nc.scalar.sqrt(sq, s1)
    nc.vector.reciprocal_approx_fast(s1.opt(), sq.opt())
    nrm = tl("nrm", bufs=2)
    _mul(nc, nrm, bnrm, s1[:, :, :, 0:1].to_broadcast(sh))

    # depth = |origin - bpos| (origin = bw cols 33:36, contiguous)
    dv = tl("dv")
    _sub(nc, dv, bpos, bw[:, :nn, :, 33:36])
    _mul(nc, t3, dv, dv)
    nc.vector.tensor_reduce(out=s1, in_=t3, axis=red, op=mybir.AluOpType.add)
    dep = tl("dep", 1, bufs=2)
    nc.scalar.sqrt(dep, s1)

    nc.scalar.dma_start(d_pos, bpos)
    nc.scalar.dma_start(d_nrm, nrm)
    nc.scalar.dma_start(d_scr, scr)
    nc.scalar.dma_start(d_dep, dep[:, :, :, 0])


_PROG_CACHE = {}


def _get_program(cfg, meta):
    key = (tuple(meta["p5_plan"]), meta["nd_cols"])
    if key not in _PROG_CACHE:
        _PROG_CACHE[key] = build_program(cfg, meta)
    return _PROG_CACHE[key]


def run(cfg, inputs, trace=False):
    percore, meta = host_prep(cfg, inputs)
    nc = _get_program(cfg, meta)
    res = run_bass_kernel_spmd(nc, percore, core_ids=list(range(8)),
                               trace=trace)
    return res, meta


def assemble(cfg, inputs, res, meta):
    c = cfg
    fid = np.asarray(inputs["face_id"])
    out = np.zeros((5, c.NBC, c.H * c.W, 3), np.float32)
    nd_cols = meta["nd_cols"]
    for core in range(8):
        r = res.results[core]
        # slot (desc d, kk) -> grid position (p=d%128, col=d//128, kk)
        pos = r["o_pos"].reshape(P, nd_cols, KPX, 3)
        nrm = r["o_nrm"].reshape(P, nd_cols, KPX, 3)
        scr = r["o_scr"].reshape(P, nd_cols, KPX, 3)
        dep = r["o_dep"].reshape(P, nd_cols, KPX)
        for (sel, bcv, didx, kk) in meta["px"][core]:
            if not len(sel):
                continue
            p, col = didx % P, didx // P
            out[1, bcv, sel] = pos[p, col, kk]
            out[2, bcv, sel] = nrm[p, col, kk]
            out[4, bcv, sel] = scr[p, col, kk]
            out[3, bcv, sel] = dep[p, col, kk][:, None]
    out = out.reshape(5, c.B, c.C, c.H, c.W, 3)
    m = (np.asarray(fid) >= 0).astype(np.float32)
    out[0] = np.broadcast_to(m[:, :, :, :, None], out[0].shape)
    return out.astype(np.float32)


def kernel(**inputs):
    cfg = Cfg()
    res, meta = run(cfg, inputs)
    return assemble(cfg, inputs, res, meta)
